# revision 1
# baseline (speedup 1.0000x reference)
"""Causal attention (B=4, S=2048, D=1024) on 8 Trainium2 NeuronCores.

Sharding: core c = (batch b = c//2, half h = c%2). Each core computes the
full attention output for 1024 query rows (rows [h*1024, (h+1)*1024) of
batch b), against the full 2048-key sequence of that batch.

Per-core kernel (SPMD, identical program, per-core data):
  Phase A: project Q^T, K^T (fp32, SBUF-resident) and V (bf16,
           SBUF-resident) from host-pre-transposed x^T and W^T inputs.
  Phase B: transposed-scores flash attention: S^T = K @ Q^T (fp32r
           matmuls), additive mask bias from the actual mask input,
           exp on ScalarE (no max subtraction; scores are ~N(0,1) by
           construction so exp is safe), P in bf16, O = P @ V and
           l = P^T-column sums accumulated in PSUM across all 16
           key blocks, then normalize O by 1/l and DMA out.
"""

import sys

sys.path.insert(0, "/opt/trn_rl_repo")

import numpy as np

import concourse.bass as bass
import concourse.mybir as mybir
from concourse import tile
from concourse.bass_utils import run_bass_kernel_spmd

F32 = mybir.dt.float32
F32R = mybir.dt.float32r
BF16 = mybir.dt.bfloat16
AF = mybir.ActivationFunctionType

B, S, D = 4, 2048, 1024
NQ = 1024          # query rows per core
NKB = 16           # key blocks of 128
NQC = 4            # query column chunks per core
QW = 256           # query width of one score tile
NMC = 8            # d_model chunks of 128 (contraction)
NDC = 8            # d_k chunks of 128
NEG = -1.0e6
SCALE = 1.0 / 32.0  # 1/sqrt(D_K)


def _set_dims(s, d, nq, qw):
    global S, D, NQ, NKB, NQC, QW, NMC, NDC, SCALE
    S, D, NQ, QW = s, d, nq, qw
    NKB = S // 128
    NQC = NQ // QW
    NMC = D // 128
    NDC = D // 128
    SCALE = 1.0 / float(np.sqrt(D))


def _build_nc():
    nc = bass.Bass()
    xqT = nc.declare_dram_parameter("xqT", [D, NQ], F32R, isOutput=False)
    xkvT = nc.declare_dram_parameter("xkvT", [D, S], F32R, isOutput=False)
    wqT = nc.declare_dram_parameter("wqT", [D, D], F32R, isOutput=False)
    wkT = nc.declare_dram_parameter("wkT", [D, D], F32R, isOutput=False)
    wvT = nc.declare_dram_parameter("wvT", [D, D], F32R, isOutput=False)
    biasT = nc.declare_dram_parameter("biasT", [NQC, NKB, 128, QW], F32, isOutput=False)
    out = nc.declare_dram_parameter("out", [NQ, D], F32, isOutput=True)

    with tile.TileContext(nc) as tc:
        with tc.tile_pool(name="res", bufs=1) as res, \
             tc.tile_pool(name="rawp", bufs=3) as rawp, \
             tc.tile_pool(name="psp", bufs=1, space="PSUM") as psp:
            # Resident: Q^T [p=dk, dc, q]; K^T [p=dk, dc, k]; V [p=k, kb, dv].
            qt_sb = res.tile([128, NDC * NQ], F32R, name="qt_sb")
            kt_sb = res.tile([128, NDC * S], F32R, name="kt_sb")
            v_sb = res.tile([128, NKB * D], BF16, name="v_sb")
            ones = res.tile([128, 1], BF16, name="ones")
            nc.vector.memset(ones[:], 1.0)

            def stage(dst_ap, dram_ap, ncols, nchunks, name):
                # DMA dram chunks into a raw tile, then one DVE copy -> dst.
                # Keeps every matmul input produced by DVE (1-wait rule).
                raw = rawp.tile([128, ncols * nchunks], F32R, name=name, tag="raw")
                for i in range(nchunks):
                    nc.sync.dma_start(
                        raw[:, i * ncols:(i + 1) * ncols], dram_ap(i)
                    )
                nc.vector.tensor_copy(dst_ap, raw[:])

            # ---------------- Phase A1: Q^T = Wq @ x_q^T ----------------
            with (
                tc.tile_pool(name="xqp", bufs=1) as xqp,
                tc.tile_pool(name="wqp", bufs=2) as wqp,
            ):
                xq_sb = xqp.tile([128, NMC * NQ], F32R, name="xq_sb")
                for mc in range(NMC):
                    stage(
                        xq_sb[:, mc * NQ:(mc + 1) * NQ],
                        lambda i, mc=mc: xqT[mc * 128:(mc + 1) * 128, :],
                        NQ, 1, "rxq",
                    )
                for dc in range(NDC):
                    wsl = wqp.tile([128, NMC * 128], F32R, name="wsl", tag="wsl")
                    stage(
                        wsl[:],
                        lambda i, dc=dc: wqT[i * 128:(i + 1) * 128, dc * 128:(dc + 1) * 128],
                        128, NMC, "rwq",
                    )
                    W1 = min(512, NQ)
                    for q2 in range(NQ // W1):
                        ps = psp.tile([128, 512], F32, name="psq", tag="st", bufs=2)[:, :W1]
                        for mc in range(NMC):
                            nc.tensor.matmul(
                                ps[:],
                                lhsT=wsl[:, mc * 128:(mc + 1) * 128],
                                rhs=xq_sb[:, mc * NQ + q2 * W1: mc * NQ + q2 * W1 + W1],
                                start=(mc == 0),
                                stop=(mc == NMC - 1),
                            )
                        nc.vector.tensor_copy(
                            qt_sb[:, dc * NQ + q2 * W1: dc * NQ + q2 * W1 + W1], ps[:]
                        )

            # ---------------- Phase A2: K^T = Wk @ x_kv^T ----------------
            with (
                tc.tile_pool(name="wkp", bufs=1) as wkp,
                tc.tile_pool(name="xcp", bufs=2) as xcp,
            ):
                wk_sb = wkp.tile([128, NMC * D], F32R, name="wk_sb")
                for mc in range(NMC):
                    stage(
                        wk_sb[:, mc * D:(mc + 1) * D],
                        lambda i, mc=mc: wkT[mc * 128:(mc + 1) * 128, :],
                        D, 1, "rwk",
                    )
                for _pi in range(2):
                    _pt = psp.tile([128, 512], F32, name=f"prime{_pi}", tag="st", bufs=2)
                    nc.vector.memset(_pt[:], 0.0)
                for kc in range(S // 256):  # 256-wide key column chunks
                    xcol = xcp.tile([128, NMC * 256], F32R, name="xcol", tag="xcol")
                    stage(
                        xcol[:],
                        lambda i, kc=kc: xkvT[i * 128:(i + 1) * 128, kc * 256:(kc + 1) * 256],
                        256, NMC, "rxc",
                    )
                    for dc in range(NDC):
                        ps = psp.tile([128, 512], F32, name="psk", tag="st", bufs=2)[:, :256]
                        for mc in range(NMC):
                            nc.tensor.matmul(
                                ps[:],
                                lhsT=wk_sb[:, mc * D + dc * 128: mc * D + dc * 128 + 128],
                                rhs=xcol[:, mc * 256:(mc + 1) * 256],
                                start=(mc == 0),
                                stop=(mc == NMC - 1),
                            )
                        nc.vector.tensor_copy(
                            kt_sb[:, dc * S + kc * 256: dc * S + kc * 256 + 256], ps[:]
                        )

            # ---------------- Phase A3: V = x_kv @ Wv^T (bf16) ----------------
            with (
                tc.tile_pool(name="wvp", bufs=1) as wvp,
                tc.tile_pool(name="xcp2", bufs=2) as xcp,
            ):
                wv_sb = wvp.tile([128, NMC * D], F32R, name="wv_sb")
                for mc in range(NMC):
                    stage(
                        wv_sb[:, mc * D:(mc + 1) * D],
                        lambda i, mc=mc: wvT[mc * 128:(mc + 1) * 128, :],
                        D, 1, "rwv",
                    )
                for _pi in range(2):
                    _pt = psp.tile([128, 512], F32, name=f"prime{_pi}", tag="st", bufs=2)
                    nc.vector.memset(_pt[:], 0.0)
                for kc in range(S // 256):
                    xcol = xcp.tile([128, NMC * 256], F32R, name="xcol2", tag="xcol2")
                    stage(
                        xcol[:],
                        lambda i, kc=kc: xkvT[i * 128:(i + 1) * 128, kc * 256:(kc + 1) * 256],
                        256, NMC, "rxc2",
                    )
                    DV = min(512, D)
                    for kbl in range(2):
                        kb = kc * 2 + kbl
                        for dvc in range(D // DV):
                            ps = psp.tile([128, 512], F32, name="psv", tag="st", bufs=2)[:, :DV]
                            for mc in range(NMC):
                                nc.tensor.matmul(
                                    ps[:],
                                    lhsT=xcol[:, mc * 256 + kbl * 128: mc * 256 + kbl * 128 + 128],
                                    rhs=wv_sb[:, mc * D + dvc * DV: mc * D + dvc * DV + DV],
                                    start=(mc == 0),
                                    stop=(mc == NMC - 1),
                                )
                            nc.vector.tensor_copy(
                                v_sb[:, kb * D + dvc * DV: kb * D + dvc * DV + DV],
                                ps[:],
                            )

            # ---------------- Phase B: attention ----------------
            with (
                tc.tile_pool(name="bp", bufs=3) as bp,
                tc.tile_pool(name="sfp", bufs=3) as sfp,
                tc.tile_pool(name="pap", bufs=3) as pap,
                tc.tile_pool(name="pep", bufs=3) as pep,
                tc.tile_pool(name="otp", bufs=2) as otp,
                tc.tile_pool(name="rcp", bufs=2) as rcp,
            ):
                for _pi in range(2):
                    _pt = psp.tile([128, 512], F32, name=f"prime{_pi}", tag="st", bufs=2)
                    nc.vector.memset(_pt[:], 0.0)
                NQB = QW // 128
                DV = min(512, D)
                NDV = D // DV
                for qc in range(NQC):
                    o_ps = [
                        psp.tile([128, DV], F32, name=f"o_ps{i}", tag=f"o{i}")
                        for i in range(NQB * NDV)
                    ]
                    l_ps = [
                        psp.tile([128, 1], F32, name=f"l_ps{qb}", tag=f"l{qb}")
                        for qb in range(NQB)
                    ]
                    for j in range(NKB):
                        st = psp.tile([128, 512], F32, name="st", tag="st", bufs=2)[:, :QW]
                        for dc in range(NDC):
                            nc.tensor.matmul(
                                st[:],
                                lhsT=kt_sb[:, dc * S + j * 128: dc * S + j * 128 + 128],
                                rhs=qt_sb[:, dc * NQ + qc * QW: dc * NQ + qc * QW + QW],
                                start=(dc == 0),
                                stop=(dc == NDC - 1),
                            )
                        bt = bp.tile([128, QW], F32, name="bt", tag="bt")
                        nc.sync.dma_start(bt[:], biasT[qc, j])
                        sf = sfp.tile([128, QW], F32, name="sf", tag="sf")
                        nc.vector.tensor_add(sf[:], st[:], bt[:])
                        pa = pap.tile([128, QW], BF16, name="pa", tag="pa")
                        nc.scalar.activation(pa[:], sf[:], AF.Exp, scale=SCALE)
                        pe = pep.tile([128, QW], BF16, name="pe", tag="pe")
                        nc.vector.tensor_copy(pe[:], pa[:])
                        for qb in range(NQB):
                            nc.tensor.matmul(
                                l_ps[qb][:],
                                lhsT=pe[:, qb * 128:(qb + 1) * 128],
                                rhs=ones[:],
                                start=(j == 0),
                                stop=(j == NKB - 1),
                            )
                            for dvc in range(NDV):
                                nc.tensor.matmul(
                                    o_ps[qb * NDV + dvc][:],
                                    lhsT=pe[:, qb * 128:(qb + 1) * 128],
                                    rhs=v_sb[:, j * D + dvc * DV: j * D + dvc * DV + DV],
                                    start=(j == 0),
                                    stop=(j == NKB - 1),
                                )
                    for qb in range(NQB):
                        rc = rcp.tile([128, 1], F32, name="rc", tag="rc")
                        nc.vector.reciprocal(rc[:], l_ps[qb][:])
                        for dvc in range(NDV):
                            ot = otp.tile([128, DV], F32, name="ot", tag="ot")
                            nc.vector.tensor_scalar_mul(
                                ot[:], o_ps[qb * NDV + dvc][:], rc[:]
                            )
                            nc.sync.dma_start(
                                out[
                                    qc * QW + qb * 128: qc * QW + qb * 128 + 128,
                                    dvc * DV: dvc * DV + DV,
                                ],
                                ot[:],
                            )
    _elide_transitive_waits(nc)
    return nc


def _elide_transitive_waits(nc):
    """Drop semaphore waits already implied transitively.

    Hardware matmul (fused LDWEIGHTS) and DMA instruction encodings accept
    only ONE sync wait.  Tile's wait assignment is per-proc minimal but NOT
    transitive, so phase boundaries emit multi-wait matmuls/DMAs.  This pass
    walks the scheduled program (list order is a valid linearization),
    maintains a transitive vector clock per proc (engines and DMA queues are
    each FIFO), and removes waits that are (a) on the instruction's own proc
    (FIFO completion order), or (b) already implied by an earlier retained
    wait's transitive closure.
    """
    import re
    _proc_re = re.compile(r"^(PE|DVE|ACT|Act|Activation|SP|Pool|POOL|DMAHW\d+|DMASW\d+)_")

    def _is_proc_sem(name):
        return bool(_proc_re.match(name or ""))

    hist = {}      # sem id -> list of (tick, snapshot dict)
    state = {}     # proc key -> dict(sem id -> observed tick)
    tickc = {}     # sem id -> cumulative tick

    def snap_at(sem, t):
        h = hist.get(sem)
        if not h:
            return None
        lo, hi, best = 0, len(h) - 1, None
        while lo <= hi:
            mid = (lo + hi) // 2
            if h[mid][0] <= t:
                best = h[mid][1]
                lo = mid + 1
            else:
                hi = mid - 1
        return best

    splits = []
    for blk in nc.m.functions[0].blocks:
        for idx, i in enumerate(blk.instructions):
            si = i.sync_info
            if si is None:
                continue
            ups = [u for u in si.on_update if _is_proc_sem(u.ant_name)]
            own = ups[0].id if ups else ("eng", str(i.engine))
            v = state.setdefault(own, {})
            keep = []
            for w in list(si.on_wait):
                if (
                    w.wait_mode != "sem-ge-imm"
                    or w.wait_reg is not None
                    or not _is_proc_sem(w.ant_name)
                ):
                    keep.append(w)
                    continue
                # Same-proc elision is ONLY safe for PE matmuls: the PE
                # completes matmuls strictly in order (pc-monotone ends), so
                # a PE-self completion wait is redundant.  Other engines have
                # deep pipelines where same-engine WAR/WAW needs the wait.
                pe_self = (
                    w.id == own
                    and type(i).__name__ in ("InstMatmult", "InstLdweights")
                    and w.ant_name.startswith("PE")
                )
                if pe_self or v.get(w.id, 0) >= w.wait_value:
                    continue  # implied: PE FIFO or transitive closure
                keep.append(w)
                v[w.id] = max(v.get(w.id, 0), w.wait_value)
                s = snap_at(w.id, w.wait_value)
                if s:
                    for k2, t2 in s.items():
                        if v.get(k2, 0) < t2:
                            v[k2] = t2
            if len(keep) > 1 and all(_is_proc_sem(w.ant_name) for w in keep):
                # Hardware instruction encodings here accept at most one
                # sync wait: hoist all waits onto standalone sequencer
                # event-semaphore wait ops inserted just before.
                for k, w in enumerate(keep):
                    splits.append(
                        (blk, idx, mybir.InstEventSemaphore(
                            name=f"{i.name}-w{k}",
                            engine=i.engine,
                            sync_info=mybir.SyncInfo(on_wait=[w], on_update=[]),
                        ))
                    )
                keep = []
            if len(keep) != len(si.on_wait):
                si.on_wait = keep
                i.sync_info = si
            for u in ups:
                inc = u.update_value if u.update_mode in ("sem-inc", "sem-add-imm") else 0
                t = tickc.get(u.id, 0) + (inc or 0)
                tickc[u.id] = t
                snapshot = dict(v)
                snapshot[u.id] = t
                hist.setdefault(u.id, []).append((t, snapshot))
            nm = type(i).__name__
            if nm in ("InstMatmult", "InstDMACopy", "InstTensorCopy",
                      "InstTensorTensor", "InstActivation", "InstMemset",
                      "InstTensorScalarPtr", "InstReciprocal", "InstLdweights"):
                assert len(i.sync_info.on_wait) <= 1, (
                    i.name, nm,
                    [(w.ant_name, w.wait_value) for w in i.sync_info.on_wait],
                )
    by_blk = {}
    for blk, idx, inst in splits:
        by_blk.setdefault(id(blk), (blk, []))[1].append((idx, inst))
    for blk, items in by_blk.values():
        for idx, inst in sorted(items, key=lambda t: -t[0]):
            nc.register_instruction(inst)
            blk.instructions.insert(idx, inst)


_CACHE = {}


def _get_nc():
    if "nc" not in _CACHE:
        _CACHE["nc"] = _build_nc()
    return _CACHE["nc"]


def make_in_maps(x, mask, Wq, Wk, Wv):
    x = np.asarray(x, dtype=np.float32)
    mask = np.asarray(mask)
    wqT = np.ascontiguousarray(np.asarray(Wq, np.float32).T)
    wkT = np.ascontiguousarray(np.asarray(Wk, np.float32).T)
    wvT = np.ascontiguousarray(np.asarray(Wv, np.float32).T)
    in_maps = []
    for c in range(8):
        b, h = divmod(c, 2)
        xb = x[b]
        xqT = np.ascontiguousarray(xb[h * NQ:(h + 1) * NQ].T)
        xkvT = np.ascontiguousarray(xb.T)
        mb = mask[b, h * NQ:(h + 1) * NQ, :]  # [1024 q, 2048 k]
        mt = mb.T.reshape(NKB, 128, NQC, QW).transpose(2, 0, 1, 3)
        bias = np.where(mt, np.float32(0.0), np.float32(NEG))
        in_maps.append(
            dict(
                xqT=xqT,
                xkvT=xkvT,
                wqT=wqT,
                wkT=wkT,
                wvT=wvT,
                biasT=np.ascontiguousarray(bias),
            )
        )
    return in_maps


def assemble(results):
    out = np.empty((B, S, D), np.float32)
    for c in range(8):
        b, h = divmod(c, 2)
        out[b, h * NQ:(h + 1) * NQ] = results[c]["out"]
    return out


def kernel(x, mask, Wq, Wk, Wv):
    nc = _get_nc()
    in_maps = make_in_maps(x, mask, Wq, Wk, Wv)
    res = run_bass_kernel_spmd(nc, in_maps, list(range(8)))
    return assemble(res.results)



# revision 6
# speedup vs baseline: 438.8941x; 438.8941x over previous
"""Causal attention (B=4, S=2048, D=1024) on 8 Trainium2 NeuronCores.

Sharding: core c = (batch b = c//2, half h = c%2). Each core computes the
full attention output for 1024 query rows of batch b, chosen as four
256-row chunks qc=0..3 at global offsets g0 = qc*512 + h*256.  This makes
the causal work identical across cores: chunk qc needs exactly the first
(4*qc+4) key blocks of 128 (rounded up from its causal extent), so every
core runs the same 4+8+12+16 = 40 block-passes -- a uniform SPMD program.

Algebraic refactoring (kills the duplicated K/V projections entirely):
  scores = Q K^T = (x Wq^T)(x Wk^T)^T = x (Wq^T Wk) x^T
     -> host folds M = Wq^T @ Wk once; device computes Q'^T = M^T x_q^T
        (one 1024-row projection) and contracts against resident X^T.
  out = P V = P (x Wv^T) = (P x) Wv^T
     -> flash loop accumulates PX^T = X^T P^T in PSUM; a single
        (PX) @ Wv^T projection per query chunk finishes the job.

Per-core kernel phases:
  A: q2t = M^T @ x_q^T (bf16 in, fp32 PSUM, bf16 out), 128 matmuls.
  B: per (qc, j): st = X^T_j q2t_qc (8 MMs), pe = exp(st/32) * mask
     (ACT + DVE, multiplicative bf16 mask from the real mask input),
     PX^T += X_j^T-chunked MMs (8), l += ones MMs (2).  PX^T packs two
     [128,256] fp32 accumulators per PSUM bank (single start=True on the
     bank's first write, single stop=True on its last -- has_written
     clears are bank-wide).  Consumer MMs are software-pipelined one
     pass behind the score MMs so the PE never waits on ACT latency.
  C: per qc: PX^T -> bf16 SBUF, out = (PX Wv^T) / l, DMA out.
"""

import sys

sys.path.insert(0, "/opt/trn_rl_repo")

import numpy as np
import ml_dtypes

import concourse.bass as bass
import concourse.mybir as mybir
from concourse import tile
from concourse.bass_utils import run_bass_kernel_spmd

F32 = mybir.dt.float32
BF16 = mybir.dt.bfloat16
AF = mybir.ActivationFunctionType
BF = ml_dtypes.bfloat16

B, S, D = 4, 2048, 1024
NQ = 1024            # query rows per core
QW = 256             # query width of one score tile
NQC = NQ // QW       # 4 query chunks per core
NMC = D // 128       # contraction chunks
NKB = S // 128       # key blocks
PASS_NKB = [4 * qc + 4 for qc in range(NQC)]   # uniform causal schedule
NPASS = sum(PASS_NKB)                          # 40
SCALE = 1.0 / 32.0   # 1/sqrt(d_k)


def _g0(h, qc):
    """Global query offset of chunk qc on half h (balanced causal split)."""
    return qc * 512 + h * 256


def _build_nc(reps=1):
    nc = bass.Bass()
    xqT = nc.declare_dram_parameter("xqT", [D, NQ], BF16, isOutput=False)
    mT = nc.declare_dram_parameter("mT", [D, D], BF16, isOutput=False)
    xT = nc.declare_dram_parameter("xT", [D, S], BF16, isOutput=False)
    xr = nc.declare_dram_parameter("xr", [S, D], BF16, isOutput=False)
    wvT = nc.declare_dram_parameter("wvT", [D, D], BF16, isOutput=False)
    dmw = nc.declare_dram_parameter("dmw", [128, NPASS * QW], BF16, isOutput=False)
    out = nc.declare_dram_parameter("out", [NQ, D], F32, isOutput=True)

    with tile.TileContext(nc) as tc:
        with tc.tile_pool(name="res", bufs=1) as res, \
             tc.tile_pool(name="pap", bufs=3) as pap, \
             tc.tile_pool(name="pep", bufs=3) as pep, \
             tc.tile_pool(name="pxp", bufs=2) as pxp, \
             tc.tile_pool(name="otp", bufs=2) as otp, \
             tc.tile_pool(name="rcp", bufs=2) as rcp, \
             tc.tile_pool(name="psp", bufs=1, space="PSUM") as psp, \
             tc.tile_pool(name="pha", bufs=1) as pha:
            # Resident SBUF tensors (all bf16).
            xT_sb = res.tile([128, NMC * S], BF16, name="xT_sb")
            xr_sb = res.tile([128, NKB * D], BF16, name="xr_sb")
            q2t_sb = res.tile([128, NMC * NQ], BF16, name="q2t_sb")
            wv_sb = res.tile([128, NMC * D], BF16, name="wv_sb")
            dm_sb = res.tile([128, NPASS * QW], BF16, name="dm_sb")
            ones = res.tile([128, 1], BF16, name="ones")
            nc.vector.memset(ones[:], 1.0)

            for _rep in range(reps):
                _emit_body(nc, tc, pha, res, pap, pep, pxp, otp, rcp, psp,
                           xT_sb, xr_sb, q2t_sb, wv_sb, dm_sb, ones,
                           xqT, mT, xT, xr, wvT, dmw, out)
    _elide_transitive_waits(nc)
    return nc


def _emit_body(nc, tc, pha, res, pap, pep, pxp, otp, rcp, psp,
               xT_sb, xr_sb, q2t_sb, wv_sb, dm_sb, ones,
               xqT, mT, xT, xr, wvT, dmw, out):
            # ---------------- Phase A: Q'^T = M^T @ x_q^T ----------------
            if True:
                xq_sb = pha.tile([128, NMC * NQ], BF16, name="xq_sb", tag="xq_sb")
                mt_sb = pha.tile([128, NMC * D], BF16, name="mt_sb", tag="mt_sb")
                for mc in range(NMC):
                    nc.sync.dma_start(
                        xq_sb[:, mc * NQ:(mc + 1) * NQ],
                        xqT[mc * 128:(mc + 1) * 128, :],
                    )
                    nc.sync.dma_start(
                        mt_sb[:, mc * D:(mc + 1) * D],
                        mT[mc * 128:(mc + 1) * 128, :],
                    )
                # Resident-tensor DMAs (consumed by phase B/C) stream in
                # behind the phase-A operands.
                nc.sync.dma_start(dm_sb[:], dmw[:, :])
                for mc in range(NMC):
                    nc.sync.dma_start(
                        xT_sb[:, mc * S:(mc + 1) * S],
                        xT[mc * 128:(mc + 1) * 128, :],
                    )
                for kb in range(NKB):
                    nc.sync.dma_start(
                        xr_sb[:, kb * D:(kb + 1) * D],
                        xr[kb * 128:(kb + 1) * 128, :],
                    )
                for mc in range(NMC):
                    nc.sync.dma_start(
                        wv_sb[:, mc * D:(mc + 1) * D],
                        wvT[mc * 128:(mc + 1) * 128, :],
                    )

                for dc in range(NMC):
                    for q2 in range(NQ // 512):
                        ps = psp.tile([128, 512], F32, name="psa", tag="proj", bufs=2)
                        for mc in range(NMC):
                            nc.tensor.matmul(
                                ps[:],
                                lhsT=mt_sb[:, mc * D + dc * 128: mc * D + dc * 128 + 128],
                                rhs=xq_sb[:, mc * NQ + q2 * 512: mc * NQ + q2 * 512 + 512],
                                start=(mc == 0),
                                stop=(mc == NMC - 1),
                            )
                        nc.vector.tensor_copy(
                            q2t_sb[:, dc * NQ + q2 * 512: dc * NQ + q2 * 512 + 512],
                            ps[:],
                        )

            # ---------------- Phase B + C: attention ----------------
            # Software pipeline: consumer (PX^T / l) matmuls for pass p are
            # emitted after the score matmuls of pass p+1, so the PE has
            # ~1us of queued work while ACT computes exp(p+1).
            pending = None          # () -> None, emits consumer MMs
            pendingC = None         # () -> None, emits phase C for prior qc

            def consume(pxt, l_ps, pe, j, nkb):
                def emit():
                    for dc in range(NMC):
                        nc.tensor.matmul(
                            pxt[dc // 2][:, (dc % 2) * QW:(dc % 2) * QW + QW],
                            lhsT=xr_sb[:, j * D + dc * 128: j * D + dc * 128 + 128],
                            rhs=pe[:],
                            start=(j == 0 and dc % 2 == 0),
                            stop=(j == nkb - 1 and dc % 2 == 1),
                        )
                    for qb in range(QW // 128):
                        nc.tensor.matmul(
                            l_ps[:, qb:qb + 1],
                            lhsT=pe[:, qb * 128:(qb + 1) * 128],
                            rhs=ones[:],
                            start=(j == 0 and qb == 0),
                            stop=(j == nkb - 1 and qb == 1),
                        )
                return emit

            def phase_c(qc, pxt, l_ps):
                def emit():
                    pxsb = pxp.tile([128, NMC * QW], BF16, name="pxsb", tag="pxsb")
                    for dc in range(NMC):
                        nc.vector.tensor_copy(
                            pxsb[:, dc * QW:(dc + 1) * QW],
                            pxt[dc // 2][:, (dc % 2) * QW:(dc % 2) * QW + QW],
                        )
                    rc = rcp.tile([128, 2], F32, name="rc", tag="rc")
                    nc.vector.reciprocal(rc[:], l_ps[:])
                    for qb in range(QW // 128):
                        for dvc in range(D // 512):
                            ops = psp.tile([128, 512], F32, name="psc", tag="proj", bufs=2)
                            for dc in range(NMC):
                                nc.tensor.matmul(
                                    ops[:],
                                    lhsT=pxsb[:, dc * QW + qb * 128: dc * QW + qb * 128 + 128],
                                    rhs=wv_sb[:, dc * D + dvc * 512: dc * D + dvc * 512 + 512],
                                    start=(dc == 0),
                                    stop=(dc == NMC - 1),
                                )
                            ot = otp.tile([128, 512], F32, name="ot", tag="ot")
                            nc.vector.tensor_scalar_mul(ot[:], ops[:], rc[:, qb:qb + 1])
                            nc.sync.dma_start(
                                out[
                                    qc * QW + qb * 128: qc * QW + qb * 128 + 128,
                                    dvc * 512: dvc * 512 + 512,
                                ],
                                ot[:],
                            )
                return emit

            p = 0
            for qc in range(NQC):
                nkb = PASS_NKB[qc]
                pxt = [
                    psp.tile([128, 512], F32, name=f"pxt{i}", tag=f"pxt{i}")
                    for i in range(4)
                ]
                l_ps = psp.tile([128, 2], F32, name="l_ps", tag="l")
                for j in range(nkb):
                    st = psp.tile([128, 512], F32, name="st", tag="st")[:, :QW]
                    for dc in range(NMC):
                        nc.tensor.matmul(
                            st[:],
                            lhsT=xT_sb[:, dc * S + j * 128: dc * S + j * 128 + 128],
                            rhs=q2t_sb[:, dc * NQ + qc * QW: dc * NQ + qc * QW + QW],
                            start=(dc == 0),
                            stop=(dc == NMC - 1),
                        )
                    pa = pap.tile([128, QW], BF16, name="pa", tag="pa")
                    nc.scalar.activation(pa[:], st[:], AF.Exp, scale=SCALE)
                    pe = pep.tile([128, QW], BF16, name="pe", tag="pe")
                    nc.vector.tensor_mul(
                        pe[:], pa[:], dm_sb[:, p * QW:(p + 1) * QW]
                    )
                    if pending is not None:
                        pending()
                    if pendingC is not None:
                        pendingC()
                        pendingC = None
                    pending = consume(pxt, l_ps, pe, j, nkb)
                    p += 1
                pending()
                pending = None
                pendingC = phase_c(qc, pxt, l_ps)
            pendingC()


def _elide_transitive_waits(nc):
    """Drop semaphore waits already implied transitively.

    Hardware matmul (fused LDWEIGHTS) and DMA instruction encodings accept
    only ONE sync wait.  Tile's wait assignment is per-proc minimal but NOT
    transitive, so phase boundaries emit multi-wait matmuls/DMAs.  This pass
    walks the scheduled program (list order is a valid linearization),
    maintains a transitive vector clock per proc (engines and DMA queues are
    each FIFO), and removes waits that are (a) on the instruction's own proc
    (FIFO completion order), or (b) already implied by an earlier retained
    wait's transitive closure.
    """
    import re
    _proc_re = re.compile(r"^(PE|DVE|ACT|Act|Activation|SP|Pool|POOL|DMAHW\d+|DMASW\d+)_")

    def _is_proc_sem(name):
        return bool(_proc_re.match(name or ""))

    hist = {}      # sem id -> list of (tick, snapshot dict)
    state = {}     # proc key -> dict(sem id -> observed tick)
    tickc = {}     # sem id -> cumulative tick

    def snap_at(sem, t):
        h = hist.get(sem)
        if not h:
            return None
        lo, hi, best = 0, len(h) - 1, None
        while lo <= hi:
            mid = (lo + hi) // 2
            if h[mid][0] <= t:
                best = h[mid][1]
                lo = mid + 1
            else:
                hi = mid - 1
        return best

    splits = []
    for blk in nc.m.functions[0].blocks:
        for idx, i in enumerate(blk.instructions):
            si = i.sync_info
            if si is None:
                continue
            ups = [u for u in si.on_update if _is_proc_sem(u.ant_name)]
            own = ups[0].id if ups else ("eng", str(i.engine))
            v = state.setdefault(own, {})
            keep = []
            for w in list(si.on_wait):
                if (
                    w.wait_mode != "sem-ge-imm"
                    or w.wait_reg is not None
                    or not _is_proc_sem(w.ant_name)
                ):
                    keep.append(w)
                    continue
                # Same-proc elision is ONLY safe for PE matmuls: the PE
                # completes matmuls strictly in order (pc-monotone ends), so
                # a PE-self completion wait is redundant.  Other engines have
                # deep pipelines where same-engine WAR/WAW needs the wait.
                pe_self = (
                    w.id == own
                    and type(i).__name__ in ("InstMatmult", "InstLdweights")
                    and w.ant_name.startswith("PE")
                )
                if pe_self or v.get(w.id, 0) >= w.wait_value:
                    continue  # implied: PE FIFO or transitive closure
                keep.append(w)
                v[w.id] = max(v.get(w.id, 0), w.wait_value)
                s = snap_at(w.id, w.wait_value)
                if s:
                    for k2, t2 in s.items():
                        if v.get(k2, 0) < t2:
                            v[k2] = t2
            if len(keep) > 1 and all(_is_proc_sem(w.ant_name) for w in keep):
                # Hardware instruction encodings here accept at most one
                # sync wait: hoist all waits onto standalone sequencer
                # event-semaphore wait ops inserted just before.
                for k, w in enumerate(keep):
                    splits.append(
                        (blk, idx, mybir.InstEventSemaphore(
                            name=f"{i.name}-w{k}",
                            engine=i.engine,
                            sync_info=mybir.SyncInfo(on_wait=[w], on_update=[]),
                        ))
                    )
                keep = []
            if len(keep) != len(si.on_wait):
                si.on_wait = keep
                i.sync_info = si
            for u in ups:
                inc = u.update_value if u.update_mode in ("sem-inc", "sem-add-imm") else 0
                t = tickc.get(u.id, 0) + (inc or 0)
                tickc[u.id] = t
                snapshot = dict(v)
                snapshot[u.id] = t
                hist.setdefault(u.id, []).append((t, snapshot))
            nm = type(i).__name__
            if nm in ("InstMatmult", "InstDMACopy", "InstTensorCopy",
                      "InstTensorTensor", "InstActivation", "InstMemset",
                      "InstTensorScalarPtr", "InstReciprocal", "InstLdweights"):
                assert len(i.sync_info.on_wait) <= 1, (
                    i.name, nm,
                    [(w.ant_name, w.wait_value) for w in i.sync_info.on_wait],
                )
    by_blk = {}
    for blk, idx, inst in splits:
        by_blk.setdefault(id(blk), (blk, []))[1].append((idx, inst))
    for blk, items in by_blk.values():
        for idx, inst in sorted(items, key=lambda t: -t[0]):
            nc.register_instruction(inst)
            blk.instructions.insert(idx, inst)


_CACHE = {}


def _get_nc(reps=1):
    if reps not in _CACHE:
        _CACHE[reps] = _build_nc(reps)
    return _CACHE[reps]


def make_in_maps(x, mask, Wq, Wk, Wv):
    x = np.asarray(x, dtype=np.float32)
    mask = np.asarray(mask)
    # Folded weight: Q' = x @ (Wq^T Wk); device stores mT = Wq^T @ Wk as
    # [d_contract, d_out] lhsT chunks.
    mT = np.ascontiguousarray((np.asarray(Wq, np.float32).T
                               @ np.asarray(Wk, np.float32)).astype(BF))
    wvT = np.ascontiguousarray(np.asarray(Wv, np.float32).T.astype(BF))
    in_maps = []
    for c in range(8):
        b, h = divmod(c, 2)
        xb = x[b]
        xb_bf = xb.astype(BF)
        qrows = np.concatenate(
            [np.arange(_g0(h, qc), _g0(h, qc) + QW) for qc in range(NQC)]
        )
        xqT = np.ascontiguousarray(xb_bf[qrows].T)
        xTb = np.ascontiguousarray(xb_bf.T)
        # Multiplicative mask tiles, [k,q] orientation, one per pass.
        dmw = np.zeros((128, NPASS * QW), np.float32)
        pidx = 0
        for qc in range(NQC):
            g0 = _g0(h, qc)
            for j in range(PASS_NKB[qc]):
                blk = mask[b, g0:g0 + QW, j * 128:(j + 1) * 128]  # [q, k]
                dmw[:, pidx * QW:(pidx + 1) * QW] = blk.T.astype(np.float32)
                pidx += 1
        in_maps.append(
            dict(
                xqT=xqT,
                mT=mT,
                xT=xTb,
                xr=np.ascontiguousarray(xb_bf),
                wvT=wvT,
                dmw=np.ascontiguousarray(dmw.astype(BF)),
            )
        )
    return in_maps


def assemble(results):
    out = np.empty((B, S, D), np.float32)
    for c in range(8):
        b, h = divmod(c, 2)
        o = results[c]["out"]
        for qc in range(NQC):
            g0 = _g0(h, qc)
            out[b, g0:g0 + QW] = o[qc * QW:(qc + 1) * QW]
    return out


def kernel(x, mask, Wq, Wk, Wv):
    nc = _get_nc()
    in_maps = make_in_maps(x, mask, Wq, Wk, Wv)
    res = run_bass_kernel_spmd(nc, in_maps, list(range(8)))
    return assemble(res.results)


# revision 13
# speedup vs baseline: 475.2387x; 1.0828x over previous
"""Causal attention (B=4, S=2048, D=1024) on 8 Trainium2 NeuronCores.

Sharding: core c = (batch b = c//2, half h = c%2). Each core computes the
full attention output for 1024 query rows of batch b, chosen as four
256-row chunks qc=0..3 at global offsets g0 = qc*512 + h*256.  This makes
the causal work identical across cores: chunk qc needs exactly the first
(4*qc+4) key blocks of 128 (rounded up from its causal extent), so every
core runs the same 4+8+12+16 = 40 block-passes -- a uniform SPMD program.

Algebraic refactoring (kills the duplicated K/V projections entirely):
  scores = Q K^T = (x Wq^T)(x Wk^T)^T = x (Wq^T Wk) x^T
     -> host folds M = Wq^T @ Wk once; device computes Q'^T = M^T x_q^T
        (one 1024-row projection) and contracts against resident X^T.
  out = P V = P (x Wv^T) = (P x) Wv^T
     -> flash loop accumulates PX^T = X^T P^T in PSUM; a single
        (PX) @ Wv^T projection per query chunk finishes the job.

Per-core kernel phases:
  A: q2t = M^T @ x_q^T (bf16 in, fp32 PSUM, bf16 out), 128 matmuls.
  B: per (qc, j): st = X^T_j q2t_qc (8 MMs), pe = exp(st/32) * mask
     (ACT + DVE, multiplicative bf16 mask from the real mask input),
     PX^T += X_j^T-chunked MMs (8), l += ones MMs (2).  PX^T packs two
     [128,256] fp32 accumulators per PSUM bank (single start=True on the
     bank's first write, single stop=True on its last -- has_written
     clears are bank-wide).  Consumer MMs are software-pipelined one
     pass behind the score MMs so the PE never waits on ACT latency.
  C: per qc: PX^T -> bf16 SBUF, out = (PX Wv^T) / l, DMA out.
"""

import sys

sys.path.insert(0, "/opt/trn_rl_repo")

import numpy as np
import ml_dtypes

import concourse.bass as bass
import concourse.mybir as mybir
from concourse import tile
from concourse.bass_utils import run_bass_kernel_spmd

F32 = mybir.dt.float32
BF16 = mybir.dt.bfloat16
AF = mybir.ActivationFunctionType
BF = ml_dtypes.bfloat16

B, S, D = 4, 2048, 1024
NQ = 1024            # query rows per core
QW = 256             # query width of one score tile
NQC = NQ // QW       # 4 query chunks per core
NMC = D // 128       # contraction chunks
NKB = S // 128       # key blocks
PASS_NKB = [4 * qc + 4 for qc in range(NQC)]   # uniform causal schedule
NPASS = sum(PASS_NKB)                          # 40
SCALE = 1.0 / 32.0   # 1/sqrt(d_k)


def _g0(h, qc):
    """Global query offset of chunk qc on half h (balanced causal split)."""
    return qc * 512 + h * 256


def _build_nc(reps=1):
    nc = bass.Bass()
    xqT = nc.declare_dram_parameter("xqT", [D, NQ], BF16, isOutput=False)
    mT = nc.declare_dram_parameter("mT", [D, D], BF16, isOutput=False)
    xT = nc.declare_dram_parameter("xT", [D, S], BF16, isOutput=False)
    xr = nc.declare_dram_parameter("xr", [S, D], BF16, isOutput=False)
    wvT = nc.declare_dram_parameter("wvT", [D, D], BF16, isOutput=False)
    # Mask tiles for the last 4 key blocks of a chunk; the pattern set is
    # chunk-independent (two diagonal triangles / ones / zeros), so only 4
    # tiles are shipped and slot j-4qc selects one.
    dmw = nc.declare_dram_parameter("dmw", [128, 4 * QW], BF16, isOutput=False)
    out = nc.declare_dram_parameter("out", [NQ, D], BF16, isOutput=True)

    with tile.TileContext(nc) as tc:
        with tc.tile_pool(name="res", bufs=1) as res, \
             tc.tile_pool(name="pap", bufs=3) as pap, \
             tc.tile_pool(name="pep", bufs=3) as pep, \
             tc.tile_pool(name="pxp", bufs=2) as pxp, \
             tc.tile_pool(name="otp", bufs=2) as otp, \
             tc.tile_pool(name="rcp", bufs=2) as rcp, \
             tc.tile_pool(name="psp", bufs=1, space="PSUM") as psp, \
             tc.tile_pool(name="pha", bufs=1) as pha:
            # Resident SBUF tensors (all bf16).
            xT_sb = res.tile([128, NMC * S], BF16, name="xT_sb")
            xr_sb = res.tile([128, NKB * D], BF16, name="xr_sb")
            q2t_sb = res.tile([128, NMC * NQ], BF16, name="q2t_sb")
            wv_sb = res.tile([128, NMC * D], BF16, name="wv_sb")
            dm_sb = res.tile([128, 4 * QW], BF16, name="dm_sb")
            ones = res.tile([128, 1], BF16, name="ones")
            nc.vector.memset(ones[:], 1.0)

            for _rep in range(reps):
                _emit_body(nc, tc, pha, res, pap, pep, pxp, otp, rcp, psp,
                           xT_sb, xr_sb, q2t_sb, wv_sb, dm_sb, ones,
                           xqT, mT, xT, xr, wvT, dmw, out)
    _elide_transitive_waits(nc)
    return nc


def _emit_body(nc, tc, pha, res, pap, pep, pxp, otp, rcp, psp,
               xT_sb, xr_sb, q2t_sb, wv_sb, dm_sb, ones,
               xqT, mT, xT, xr, wvT, dmw, out):
            # ---------------- Phase A: Q'^T = M^T @ x_q^T ----------------
            if True:
                xq_sb = pha.tile([128, NMC * NQ], BF16, name="xq_sb", tag="xq_sb")
                mt_sb = pha.tile([128, NMC * D], BF16, name="mt_sb", tag="mt_sb")
                for mc in range(NMC):
                    nc.sync.dma_start(
                        xq_sb[:, mc * NQ:(mc + 1) * NQ],
                        xqT[mc * 128:(mc + 1) * 128, :],
                    )
                    nc.sync.dma_start(
                        mt_sb[:, mc * D:(mc + 1) * D],
                        mT[mc * 128:(mc + 1) * 128, :],
                    )
                # Resident-tensor DMAs (consumed by phase B/C) stream in
                # behind the phase-A operands.
                nc.sync.dma_start(dm_sb[:], dmw[:, :])
                for mc in range(NMC):
                    nc.sync.dma_start(
                        xT_sb[:, mc * S:(mc + 1) * S],
                        xT[mc * 128:(mc + 1) * 128, :],
                    )
                for kb in range(NKB):
                    nc.sync.dma_start(
                        xr_sb[:, kb * D:(kb + 1) * D],
                        xr[kb * 128:(kb + 1) * 128, :],
                    )
                for mc in range(NMC):
                    nc.sync.dma_start(
                        wv_sb[:, mc * D:(mc + 1) * D],
                        wvT[mc * 128:(mc + 1) * 128, :],
                    )

                for q2 in range(NQ // 512):
                    for dc in range(NMC):
                        ps = psp.tile([128, 512], F32, name="psa", tag="proj", bufs=2)
                        for mc in range(NMC):
                            nc.tensor.matmul(
                                ps[:],
                                lhsT=mt_sb[:, mc * D + dc * 128: mc * D + dc * 128 + 128],
                                rhs=xq_sb[:, mc * NQ + q2 * 512: mc * NQ + q2 * 512 + 512],
                                start=(mc == 0),
                                stop=(mc == NMC - 1),
                            )
                        nc.vector.tensor_copy(
                            q2t_sb[:, dc * NQ + q2 * 512: dc * NQ + q2 * 512 + 512],
                            ps[:],
                        )

            # ---------------- Phase B + C: attention ----------------
            # Software pipeline: consumer (PX^T / l) matmuls for pass p are
            # emitted after the score matmuls of pass p+1, so the PE has
            # ~1us of queued work while ACT computes exp(p+1).
            pending = None          # () -> None, emits consumer MMs
            pendingC = None         # () -> None, emits phase C for prior qc

            def consume(pxt, l_ps, pe, j, nkb):
                def emit():
                    for dc in range(NMC):
                        nc.tensor.matmul(
                            pxt[dc // 2][:, (dc % 2) * QW:(dc % 2) * QW + QW],
                            lhsT=xr_sb[:, j * D + dc * 128: j * D + dc * 128 + 128],
                            rhs=pe[:],
                            start=(j == 0 and dc % 2 == 0),
                            stop=(j == nkb - 1 and dc % 2 == 1),
                        )
                    for qb in range(QW // 128):
                        nc.tensor.matmul(
                            l_ps[:, qb:qb + 1],
                            lhsT=pe[:, qb * 128:(qb + 1) * 128],
                            rhs=ones[:],
                            start=(j == 0 and qb == 0),
                            stop=(j == nkb - 1 and qb == 1),
                        )
                return emit

            def phase_c(qc, pxt, l_ps):
                def emit():
                    pxsb = pxp.tile([128, NMC * QW], BF16, name="pxsb", tag="pxsb")
                    for dc in range(NMC):
                        nc.vector.tensor_copy(
                            pxsb[:, dc * QW:(dc + 1) * QW],
                            pxt[dc // 2][:, (dc % 2) * QW:(dc % 2) * QW + QW],
                        )
                    rc = rcp.tile([128, 2], F32, name="rc", tag="rc")
                    nc.vector.reciprocal(rc[:], l_ps[:])
                    for qb in range(QW // 128):
                        for dvc in range(D // 512):
                            ops = psp.tile([128, 512], F32, name="psc", tag="proj", bufs=2)
                            for dc in range(NMC):
                                nc.tensor.matmul(
                                    ops[:],
                                    lhsT=pxsb[:, dc * QW + qb * 128: dc * QW + qb * 128 + 128],
                                    rhs=wv_sb[:, dc * D + dvc * 512: dc * D + dvc * 512 + 512],
                                    start=(dc == 0),
                                    stop=(dc == NMC - 1),
                                )
                            ot = otp.tile([128, 512], BF16, name="ot", tag="ot")
                            nc.vector.tensor_scalar_mul(ot[:], ops[:], rc[:, qb:qb + 1])
                            nc.sync.dma_start(
                                out[
                                    qc * QW + qb * 128: qc * QW + qb * 128 + 128,
                                    dvc * 512: dvc * 512 + 512,
                                ],
                                ot[:],
                            )
                return emit

            p = 0
            for qc in range(NQC):
                nkb = PASS_NKB[qc]
                pxt = [
                    psp.tile([128, 512], F32, name=f"pxt{i}", tag=f"pxt{i}")
                    for i in range(4)
                ]
                l_ps = psp.tile([128, 2], F32, name="l_ps", tag="l")
                for j in range(nkb):
                    st = psp.tile([128, 512], F32, name="st", tag="st")[:, :QW]
                    for dc in range(NMC):
                        nc.tensor.matmul(
                            st[:],
                            lhsT=xT_sb[:, dc * S + j * 128: dc * S + j * 128 + 128],
                            rhs=q2t_sb[:, dc * NQ + qc * QW: dc * NQ + qc * QW + QW],
                            start=(dc == 0),
                            stop=(dc == NMC - 1),
                        )
                    pa = pap.tile([128, QW], BF16, name="pa", tag="pa")
                    nc.scalar.activation(pa[:], st[:], AF.Exp, scale=SCALE)
                    if j >= 4 * qc:
                        # one of the chunk's last 4 key blocks: apply mask
                        m = j - 4 * qc
                        pe = pep.tile([128, QW], BF16, name="pe", tag="pe")
                        nc.vector.tensor_mul(
                            pe[:], pa[:], dm_sb[:, m * QW:(m + 1) * QW]
                        )
                    else:
                        pe = pa  # all-keep block on every core
                    if pending is not None:
                        pending()
                    if pendingC is not None:
                        pendingC()
                        pendingC = None
                    pending = consume(pxt, l_ps, pe, j, nkb)
                    p += 1
                pending()
                pending = None
                pendingC = phase_c(qc, pxt, l_ps)
            pendingC()


def _elide_transitive_waits(nc):
    """Drop semaphore waits already implied transitively.

    Hardware matmul (fused LDWEIGHTS) and DMA instruction encodings accept
    only ONE sync wait.  Tile's wait assignment is per-proc minimal but NOT
    transitive, so phase boundaries emit multi-wait matmuls/DMAs.  This pass
    walks the scheduled program (list order is a valid linearization),
    maintains a transitive vector clock per proc (engines and DMA queues are
    each FIFO), and removes waits that are (a) on the instruction's own proc
    (FIFO completion order), or (b) already implied by an earlier retained
    wait's transitive closure.
    """
    import re
    _proc_re = re.compile(r"^(PE|DVE|ACT|Act|Activation|SP|Pool|POOL|DMAHW\d+|DMASW\d+)_")

    def _is_proc_sem(name):
        return bool(_proc_re.match(name or ""))

    hist = {}      # sem id -> list of (tick, snapshot dict)
    state = {}     # proc key -> dict(sem id -> observed tick)
    tickc = {}     # sem id -> cumulative tick

    def snap_at(sem, t):
        h = hist.get(sem)
        if not h:
            return None
        lo, hi, best = 0, len(h) - 1, None
        while lo <= hi:
            mid = (lo + hi) // 2
            if h[mid][0] <= t:
                best = h[mid][1]
                lo = mid + 1
            else:
                hi = mid - 1
        return best

    splits = []
    for blk in nc.m.functions[0].blocks:
        for idx, i in enumerate(blk.instructions):
            si = i.sync_info
            if si is None:
                continue
            ups = [u for u in si.on_update if _is_proc_sem(u.ant_name)]
            own = ups[0].id if ups else ("eng", str(i.engine))
            v = state.setdefault(own, {})
            keep = []
            for w in list(si.on_wait):
                if (
                    w.wait_mode != "sem-ge-imm"
                    or w.wait_reg is not None
                    or not _is_proc_sem(w.ant_name)
                ):
                    keep.append(w)
                    continue
                # Same-proc elision is ONLY safe for PE matmuls: the PE
                # completes matmuls strictly in order (pc-monotone ends), so
                # a PE-self completion wait is redundant.  Other engines have
                # deep pipelines where same-engine WAR/WAW needs the wait.
                pe_self = (
                    w.id == own
                    and type(i).__name__ in ("InstMatmult", "InstLdweights")
                    and w.ant_name.startswith("PE")
                )
                if pe_self:
                    continue  # PE FIFO makes the self-wait redundant
                keep.append(w)
                v[w.id] = max(v.get(w.id, 0), w.wait_value)
                s = snap_at(w.id, w.wait_value)
                if s:
                    for k2, t2 in s.items():
                        if v.get(k2, 0) < t2:
                            v[k2] = t2
            if len(keep) > 1 and all(_is_proc_sem(w.ant_name) for w in keep):
                # Hardware instruction encodings here accept at most one
                # sync wait: hoist all waits onto standalone sequencer
                # event-semaphore wait ops inserted just before.
                for k, w in enumerate(keep):
                    splits.append(
                        (blk, idx, mybir.InstEventSemaphore(
                            name=f"{i.name}-w{k}",
                            engine=i.engine,
                            sync_info=mybir.SyncInfo(on_wait=[w], on_update=[]),
                        ))
                    )
                keep = []
            if len(keep) != len(si.on_wait):
                si.on_wait = keep
                i.sync_info = si
            for u in ups:
                inc = u.update_value if u.update_mode in ("sem-inc", "sem-add-imm") else 0
                t = tickc.get(u.id, 0) + (inc or 0)
                tickc[u.id] = t
                snapshot = dict(v)
                snapshot[u.id] = t
                hist.setdefault(u.id, []).append((t, snapshot))
            nm = type(i).__name__
            if nm in ("InstMatmult", "InstDMACopy", "InstTensorCopy",
                      "InstTensorTensor", "InstActivation", "InstMemset",
                      "InstTensorScalarPtr", "InstReciprocal", "InstLdweights"):
                assert len(i.sync_info.on_wait) <= 1, (
                    i.name, nm,
                    [(w.ant_name, w.wait_value) for w in i.sync_info.on_wait],
                )
    by_blk = {}
    for blk, idx, inst in splits:
        by_blk.setdefault(id(blk), (blk, []))[1].append((idx, inst))
    for blk, items in by_blk.values():
        for idx, inst in sorted(items, key=lambda t: -t[0]):
            nc.register_instruction(inst)
            blk.instructions.insert(idx, inst)


_CACHE = {}


def _get_nc(reps=1):
    if reps not in _CACHE:
        _CACHE[reps] = _build_nc(reps)
    return _CACHE[reps]


def make_in_maps(x, mask, Wq, Wk, Wv):
    x = np.asarray(x, dtype=np.float32)
    mask = np.asarray(mask)
    # Folded weight: Q' = x @ (Wq^T Wk); device stores mT = Wq^T @ Wk as
    # [d_contract, d_out] lhsT chunks.
    mT = np.ascontiguousarray((np.asarray(Wq, np.float32).T
                               @ np.asarray(Wk, np.float32)).astype(BF))
    wvT = np.ascontiguousarray(np.asarray(Wv, np.float32).T.astype(BF))
    in_maps = []
    for c in range(8):
        b, h = divmod(c, 2)
        xb = x[b]
        xb_bf = xb.astype(BF)
        qrows = np.concatenate(
            [np.arange(_g0(h, qc), _g0(h, qc) + QW) for qc in range(NQC)]
        )
        xqT = np.ascontiguousarray(xb_bf[qrows].T)
        xTb = np.ascontiguousarray(xb_bf.T)
        # Multiplicative mask tiles, [k,q] orientation: 4 slot tiles,
        # identical across chunks (verified against the actual mask).
        dmw = np.zeros((128, 4 * QW), np.float32)
        for qc in range(NQC):
            g0 = _g0(h, qc)
            for j in range(PASS_NKB[qc]):
                blk = mask[b, g0:g0 + QW, j * 128:(j + 1) * 128]  # [q, k]
                if j >= 4 * qc:
                    m = j - 4 * qc
                    t = blk.T.astype(np.float32)
                    if qc == 0:
                        dmw[:, m * QW:(m + 1) * QW] = t
                    else:
                        assert (dmw[:, m * QW:(m + 1) * QW] == t).all(), (
                            f"core {c} chunk {qc} slot {m}: mask not "
                            "chunk-independent"
                        )
                else:
                    assert blk.all(), (
                        f"core {c} chunk {qc} block {j} expected all-keep"
                    )
        in_maps.append(
            dict(
                xqT=xqT,
                mT=mT,
                xT=xTb,
                xr=np.ascontiguousarray(xb_bf),
                wvT=wvT,
                dmw=np.ascontiguousarray(dmw.astype(BF)),
            )
        )
    return in_maps


def assemble(results):
    out = np.empty((B, S, D), np.float32)
    for c in range(8):
        b, h = divmod(c, 2)
        o = np.asarray(results[c]["out"], dtype=np.float32)
        for qc in range(NQC):
            g0 = _g0(h, qc)
            out[b, g0:g0 + QW] = o[qc * QW:(qc + 1) * QW]
    return out


def kernel(x, mask, Wq, Wk, Wv):
    nc = _get_nc()
    in_maps = make_in_maps(x, mask, Wq, Wk, Wv)
    res = run_bass_kernel_spmd(nc, in_maps, list(range(8)))
    return assemble(res.results)
